# revision 32
# baseline (speedup 1.0000x reference)
"""Causal self-attention (RoPE) Trainium2 kernel.

Sharding: 2 batches x 16 heads = 32 (b,h) units over 8 cores -> each core
handles 1 batch x 4 heads. Column-parallel QKV + row-parallel output
projection; host sums the 4 fp16 partial outputs per batch.

All matmul operands are fp16 (1 cycle/row on the PE; fp32 PSUM accum).

Per-core structure -- one fully software-pipelined stream. The ScalarE
exp cadence (~500 ns per 512-col k-chunk) exceeds the PE's S+y matmul
work per chunk (~430 ns), so attention units are never run back-to-back:
every ATT unit is surrounded by independent PE work (QK chains of a
later t-block, or output-projection chains of an earlier q-block) that
fills the PE while ScalarE catches up.

  slot tt=0..3:                 # t-major 512-col blocks, xs ring buffer
    V(tch 4tt..4tt+3)           # V = x@Wv in [t, d] layout (ScalarE evict)
    for h in 0..3:
      QK(jc 2 of 8, tt)         # Q^T,K^T in [d, t] layout + RoPE
      ATT(h, qt=tt-1)           # interleaved, with PROJ chains popped in
  post: ATT(h, qt=3) draining PROJ(qt2), then PROJ(qt3)

  RoPE: head dims host-permuted so each 32-partition block holds
  [re pairs | im pairs]. PSUM evicted to fp16 SBUF by ScalarE, then all
  DVE ops run in 16-bit mode: v=s*sin2s, u=s*cos2 (in place),
  w=swap16(v), qkT=u+w.

  ATT per (h, q-tile 512): S^T[k,q] = (K^T chunk)^T @ Q^T computed in
  PAIRS into [128,1024] PSUM tiles (2 banks) so one exp instruction
  covers two k-chunks; no max subtraction (|logits*scale| <= ~6);
  diagonal chunks multiplied by a 0/1 fp16 mask AFTER exp (16-bit DVE).
  y^T[d,q] accumulates A@V in PSUM. Denominator: DVE tree-sum of the A
  tiles (fp16) then ONE matmul with an all-ones [128,128] stationary ->
  result lands replicated on all 128 partitions, so normalization is
  reciprocal + one multiply (no partition broadcast). The denominator
  matmul + normalization is deferred past the next unit's first S
  matmuls so the PE never waits on the DVE tree-sum.
"""

import sys

if "/opt/trn_rl_repo" not in sys.path:
    sys.path.insert(0, "/opt/trn_rl_repo")

import numpy as np

import concourse.bass as bass
import concourse.tile as tile
from concourse import bacc, mybir
from concourse.bass_utils import run_bass_kernel_spmd

F32 = mybir.dt.float32
F16 = mybir.dt.float16

B, T, C = 2, 2048, 2048
NH, HD = 16, 128
NHL = 4            # heads per core
D_LOC = NHL * HD   # 512 local head dims
N_CORES = 8
SCALE = 1.0 / float(np.sqrt(HD))

CC = C // 128      # 16 contraction chunks
KC = T // 128      # 16 key chunks
QT = 512           # q tile
NQT = T // QT      # 4 q tiles

_compiled = None


def _build():
    nc = bacc.Bacc("TRN2", target_bir_lowering=False, debug=False)

    xT_d = nc.dram_tensor("xT", [CC, 128, T], F16, kind="ExternalInput")
    wq_d = nc.dram_tensor("wq", [NHL, 128, CC, 128], F16, kind="ExternalInput")
    wk_d = nc.dram_tensor("wk", [NHL, 128, CC, 128], F16, kind="ExternalInput")
    wv_d = nc.dram_tensor("wv", [128, CC, D_LOC], F16, kind="ExternalInput")
    w2_d = nc.dram_tensor("w2", [128, NHL, C], F16, kind="ExternalInput")
    cos2_d = nc.dram_tensor("cos2", [128, T], F16, kind="ExternalInput")
    sin2s_d = nc.dram_tensor("sin2s", [128, T], F16, kind="ExternalInput")
    masks_d = nc.dram_tensor("masks", [128, 896], F16, kind="ExternalInput")
    out_d = nc.dram_tensor("out", [T, C], F16, kind="ExternalOutput")

    swap_mask = list(range(16, 32)) + list(range(16))

    with tile.TileContext(nc) as tc, \
         tc.tile_pool(name="persist", bufs=1) as persist, \
         tc.tile_pool(name="px", bufs=2) as px, \
         tc.tile_pool(name="rope", bufs=2) as prope, \
         tc.tile_pool(name="att", bufs=4) as patt, \
         tc.tile_pool(name="nrm", bufs=2) as pnrm, \
         tc.tile_pool(name="outp", bufs=3) as pout, \
         tc.tile_pool(name="ps", bufs=2, space="PSUM") as ps:
        # persistent tiles
        qkT = persist.tile([128, 8, T], F16, tag="qkT")
        v_sb = persist.tile([128, KC, D_LOC], F16, tag="vsb")
        yT = persist.tile([128, NHL, T], F16, tag="yT")
        mask01 = persist.tile([128, 896], F16, tag="masks")
        cos2 = persist.tile([128, T], F16, tag="cos2")
        sin2s = persist.tile([128, T], F16, tag="sin2s")
        w2_sb = persist.tile([128, NHL, C], F16, tag="w2")
        wv_sb = persist.tile([128, CC, D_LOC], F16, tag="wv")
        wqk = persist.tile([128, 8, CC, 128], F16, tag="wqk")
        ones_sb = persist.tile([128, 128], F16, tag="ones")
        nc.vector.memset(ones_sb, 1.0)

        # warm-up matmuls: keep the PE busy (and its p-state ramping) while
        # the first x/wv DMAs land. The first 8 cover both stp PSUM slots
        # with bounded values so the diagonal-shortened S matmuls of qt0 can
        # leave stale regions there safely; the rest are tiny dummies.
        wdum = pnrm.tile([128, QT], F16, tag="t", name="wdum")
        nc.vector.memset(wdum, 0.04)
        for s in range(2):
            wstp = ps.tile([128, 2 * QT], F32, tag="stp", name=f"wstp{s}")
            for half in range(2):
                for _ in range(2):
                    nc.tensor.matmul(
                        wstp[:, half * QT:(half + 1) * QT],
                        ones_sb, wdum, start=True, stop=True)
        warm = ps.tile([128, 128], F32, tag="dp", name="warm")
        for _ in range(68):
            nc.tensor.matmul(warm, ones_sb, ones_sb, start=True, stop=True)

        # ---- DMA emission in first-use order ----
        xs_blk = [None] * NQT

        def dma_x_block(tt):
            # one descriptor-issue for the whole 2 MB block: the sync queue
            # issues dma_starts serially at ~600 ns each, so batching matters
            xs = px.tile([128, CC, QT], F16, tag="xs", name=f"xs{tt}")
            nc.sync.dma_start(
                out=xs,
                in_=xT_d.ap()[:, :, tt * QT:(tt + 1) * QT].transpose(
                    [1, 0, 2]),
            )
            xs_blk[tt] = xs

        # first x block in interleaved cc-groups so V(tch0) starts early;
        # xs0 issued from the (idle) Scalar engine's DGE in parallel with
        # wv on the sync queue — descriptor issue is ~650 ns serial per queue
        xs0 = px.tile([128, CC, QT], F16, tag="xs", name="xs0")
        for g in range(2):
            cs = slice(8 * g, 8 * g + 8)
            nc.sync.dma_start(out=wv_sb[:, cs, :], in_=wv_d.ap()[:, cs, :])
            nc.sync.dma_start(
                out=xs0[:, cs, :256],
                in_=xT_d.ap()[cs, :, :256].transpose([1, 0, 2]))
        nc.sync.dma_start(
            out=xs0[:, :, 256:],
            in_=xT_d.ap()[:, :, 256:QT].transpose([1, 0, 2]))
        xs_blk[0] = xs0
        for jc in range(8):
            w_src = (wq_d if jc < 4 else wk_d).ap()[jc % 4]
            nc.sync.dma_start(out=wqk[:, jc], in_=w_src)
        nc.sync.dma_start(out=cos2, in_=cos2_d.ap())
        nc.sync.dma_start(out=sin2s, in_=sin2s_d.ap())
        nc.sync.dma_start(out=mask01, in_=masks_d.ap())
        dma_x_block(1)
        nc.sync.dma_start(out=w2_sb, in_=w2_d.ap())
        dma_x_block(2)
        dma_x_block(3)

        def emit_v(tt):
            xs = xs_blk[tt]
            for tl in range(4):
                tch = 4 * tt + tl
                pv = ps.tile([128, QT], F32, tag="A", name=f"pv{tch}")
                for cc in range(CC):
                    nc.tensor.matmul(
                        pv,
                        xs[:, cc, tl * 128:(tl + 1) * 128],
                        wv_sb[:, cc, :],
                        start=(cc == 0), stop=(cc == CC - 1),
                    )
                nc.scalar.copy(v_sb[:, tch, :], pv)

        def emit_qk(jc, tt):
            gt0 = tt * QT
            xs = xs_blk[tt]
            psq = ps.tile([128, QT], F32, tag="A", name=f"psq{jc}_{tt}")
            for cc in range(CC):
                nc.tensor.matmul(
                    psq, wqk[:, jc, cc, :],
                    xs[:, cc, :],
                    start=(cc == 0), stop=(cc == CC - 1),
                )
            s16 = prope.tile([128, QT], F16, tag="s16", name=f"s{jc}{tt}")
            v16 = prope.tile([128, QT], F16, tag="v16", name=f"v{jc}{tt}")
            w16 = prope.tile([128, QT], F16, tag="w16", name=f"w{jc}{tt}")
            nc.scalar.copy(s16, psq)
            nc.vector.tensor_mul(v16, s16, sin2s[:, gt0:gt0 + QT])
            nc.vector.tensor_mul(s16, s16, cos2[:, gt0:gt0 + QT])
            nc.vector.stream_shuffle(w16, v16, swap_mask)
            nc.vector.tensor_add(qkT[:, jc, gt0:gt0 + QT], s16, w16)

        # queue of output-projection chains, popped as PE filler inside
        # attention kc loops
        proj_queue = []

        def emit_proj_qc(qc, split_dma=False):
            # 4 ct chains -> one [128, C] SBUF row tile -> ONE full-row DMA
            # (split in two for the tail pops so DMA overlaps the evicts).
            # Tail pops draw PSUM from the stp ring (free once attention is
            # done) for deeper chain pipelining.
            osb = pout.tile([128, C], F16, tag="o", bufs=4,
                            name=f"osb{qc}")
            for ct in range(C // QT):
                # tail pops alternate over both free PSUM rings (4 slots)
                tail_tag = "stp" if ct % 2 == 0 else "dp"
                ops = ps.tile([128, QT], F32,
                              tag=tail_tag if split_dma else "dp",
                              bufs=2 if (split_dma and ct % 2 == 0) else None,
                              name=f"ops{qc}{ct}")
                for h in range(NHL):
                    nc.tensor.matmul(
                        ops,
                        yT[:, h, qc * 128:(qc + 1) * 128],
                        w2_sb[:, h, ct * QT:(ct + 1) * QT],
                        start=(h == 0), stop=(h == NHL - 1),
                    )
                # evicts 1:3 ACT:DVE — ScalarE's exp cadence is the tighter
                # constraint wherever these pops land
                if ct == 0:
                    nc.scalar.copy(osb[:, ct * QT:(ct + 1) * QT], ops)
                else:
                    nc.vector.tensor_copy(osb[:, ct * QT:(ct + 1) * QT], ops)
                if split_dma and ct == 1:
                    nc.sync.dma_start(
                        out=out_d.ap()[qc * 128:(qc + 1) * 128, :2 * QT],
                        in_=osb[:, :2 * QT],
                    )
            if split_dma:
                nc.sync.dma_start(
                    out=out_d.ap()[qc * 128:(qc + 1) * 128, 2 * QT:],
                    in_=osb[:, 2 * QT:],
                )
            else:
                nc.sync.dma_start(
                    out=out_d.ap()[qc * 128:(qc + 1) * 128, :],
                    in_=osb,
                )

        def pop_proj(n=1, split_dma=False):
            for _ in range(n):
                if proj_queue:
                    emit_proj_qc(proj_queue.pop(0), split_dma=split_dma)

        def queue_proj(qt):
            for qc in range(4 * qt, 4 * qt + 4):
                proj_queue.append(qc)

        # deferred denominator+normalization of the previous ATT unit
        pending_fin = [None]

        def flush_fin():
            if pending_fin[0] is not None:
                pending_fin[0]()
                pending_fin[0] = None

        def emit_att(h, qt):
            q0 = qt * QT
            nkc = 4 * qt + 4   # valid k chunks (causal)
            npair = nkc // 2
            qT_ap = qkT[:, h, q0:q0 + QT]
            yps = ps.tile([128, QT], F32, tag="A", name=f"yps{h}{qt}")
            t_acc = pnrm.tile([128, QT], F16, tag="t", name=f"t{h}{qt}")
            a_pairs = [None] * npair

            def emit_pair(p):
                stp = ps.tile([128, 2 * QT], F32, tag="stp",
                              name=f"stp{h}{qt}{p}")
                for half in range(2):
                    kc = 2 * p + half
                    # diagonal blocks: only q >= 128*o is unmasked, so
                    # stream just those columns; the stale PSUM left in the
                    # skipped region is bounded (warm-up/old S values) and
                    # zeroed by the post-exp mask.
                    o = kc - 4 * qt
                    c0 = 128 * o if o > 0 else 0
                    nc.tensor.matmul(
                        stp[:, half * QT + c0:(half + 1) * QT],
                        qkT[:, 4 + h, kc * 128:(kc + 1) * 128],
                        qT_ap[:, c0:], start=True, stop=True,
                    )
                a = patt.tile([128, 2 * QT], F16, tag="a",
                              name=f"a{h}{qt}{p}")
                nc.scalar.activation(
                    a, stp, mybir.ActivationFunctionType.Exp,
                    scale=SCALE,
                )
                for half in range(2):
                    o = 2 * p + half - 4 * qt
                    if o >= 0:
                        nc.vector.tensor_mul(
                            a[:, half * QT:(half + 1) * QT],
                            a[:, half * QT:(half + 1) * QT],
                            mask01[:, 384 - 128 * o: 896 - 128 * o])
                a_pairs[p] = a

            emit_pair(0)
            if npair > 1:
                emit_pair(1)
            flush_fin()
            for kc in range(nkc):
                p, half = divmod(kc, 2)
                if half == 0 and p + 2 < npair:
                    emit_pair(p + 2)
                a_h = a_pairs[p][:, half * QT:(half + 1) * QT]
                o = kc - 4 * qt
                c0 = 128 * o if o > 0 else 0
                nc.tensor.matmul(
                    yps[:, c0:], v_sb[:, kc, h * HD:(h + 1) * HD],
                    a_h[:, c0:],
                    start=(kc == 0), stop=(kc == nkc - 1),
                    skip_group_check=True,
                )
                if kc == 1:
                    nc.vector.tensor_add(
                        t_acc, a_pairs[0][:, :QT], a_pairs[0][:, QT:])
                elif kc > 1:
                    nc.vector.tensor_add(t_acc, t_acc, a_h)
                # one qc-granular PROJ pop per unit, mid-loop; the last unit
                # holds its pop back to fill the final-normalization window
                if kc == nkc // 2 and not (qt == 3 and h == 3):
                    pop_proj()

            def fin():
                dps = ps.tile([128, QT], F32, tag="dp", name=f"dps{h}{qt}")
                nc.tensor.matmul(dps, ones_sb, t_acc, start=True, stop=True)
                rb = pnrm.tile([128, QT], F32, tag="rb", name=f"rb{h}{qt}")
                nc.vector.reciprocal_approx_fast(rb, dps)
                nc.vector.tensor_mul(yT[:, h, q0:q0 + QT], yps, rb)

            pending_fin[0] = fin

        # ---- main pipeline ----
        for tt in range(NQT):
            emit_v(tt)
            qt = tt - 1
            for h in range(NHL):
                emit_qk(h, tt)
                emit_qk(4 + h, tt)
                if qt >= 0:
                    emit_att(h, qt)
            if qt >= 0:
                queue_proj(qt)
        for h in range(NHL):
            emit_att(h, 3)
        flush_fin()
        pop_proj()   # held-back last PROJ(qt2) chain covers the fin chain
        queue_proj(3)
        pop_proj(len(proj_queue), split_dma=True)

    nc.compile()
    return nc


def _prep_core_inputs(core, x16, W_attn, W_proj, cos2, sin2s, masks):
    b = core // 4
    g = core % 4
    heads = [g * NHL + i for i in range(NHL)]
    # stream_shuffle permutes within 32-partition blocks only: lay out each
    # block as [re pairs 16b..16b+15 | im pairs 16b..16b+15]
    perm = np.concatenate(
        [np.r_[2 * (16 * blk + np.arange(16)),
               2 * (16 * blk + np.arange(16)) + 1]
         for blk in range(4)]
    )

    xT = np.ascontiguousarray(x16[b].T).reshape(CC, 128, T)

    def qk_blocks(base):
        blocks = []
        for h in heads:
            blk = W_attn[:, base + h * HD: base + (h + 1) * HD][:, perm]
            blocks.append(blk.reshape(CC, 128, HD).transpose(1, 0, 2))
        return np.ascontiguousarray(np.stack(blocks, axis=0)).astype(np.float16)

    wq = qk_blocks(0)
    wk = qk_blocks(C)
    wv = np.concatenate(
        [W_attn[:, 2 * C + h * HD: 2 * C + (h + 1) * HD] for h in heads],
        axis=1,
    )  # (C, D_LOC)
    wv = np.ascontiguousarray(
        wv.reshape(CC, 128, D_LOC).transpose(1, 0, 2)).astype(np.float16)
    w2 = np.ascontiguousarray(
        np.stack([W_proj[h * HD:(h + 1) * HD, :] for h in heads], axis=0)
        .transpose(1, 0, 2)
    ).astype(np.float16)
    return {
        "xT": xT, "wq": wq, "wk": wk, "wv": wv, "w2": w2,
        "cos2": cos2, "sin2s": sin2s, "masks": masks,
    }


def _run(inputs, trace=False):
    global _compiled
    x = np.asarray(inputs["x"], dtype=np.float32)
    W_attn = np.asarray(inputs["W_attn"], dtype=np.float32)
    W_proj = np.asarray(inputs["W_proj"], dtype=np.float32)
    fc = np.asarray(inputs["freqs_cos"], dtype=np.float32)
    fs = np.asarray(inputs["freqs_sin"], dtype=np.float32)

    x16 = x.astype(np.float16)

    cosT = np.ascontiguousarray(fc.T)            # (64, T)
    sinT = np.ascontiguousarray(fs.T)
    # per 32-partition block b: partitions [0:16] carry cos/sin of pairs
    # 16b..16b+15 (re half, +sin), [16:32] the same freqs (im half, -sin)
    cos2 = np.concatenate(
        [np.concatenate([cosT[16 * blk:16 * (blk + 1)]] * 2, axis=0)
         for blk in range(4)], axis=0)           # (128, T)
    sin2s = np.concatenate(
        [np.concatenate([sinT[16 * blk:16 * (blk + 1)],
                         -sinT[16 * blk:16 * (blk + 1)]], axis=0)
         for blk in range(4)], axis=0)
    cos2 = np.ascontiguousarray(cos2).astype(np.float16)
    sin2s = np.ascontiguousarray(sin2s).astype(np.float16)

    ki = np.arange(128)[:, None]
    u = np.arange(896)[None, :]
    masks = np.ascontiguousarray(
        np.where(ki + 384 <= u, 1.0, 0.0).astype(np.float16))  # (128, 896)

    if _compiled is None:
        _compiled = _build()
    nc = _compiled

    in_maps = [
        _prep_core_inputs(c, x16, W_attn, W_proj, cos2, sin2s, masks)
        for c in range(N_CORES)
    ]
    res = run_bass_kernel_spmd(
        nc, in_maps, core_ids=list(range(N_CORES)), trace=trace)

    out = np.zeros((B, T, C), dtype=np.float32)
    for c in range(N_CORES):
        out[c // 4] += res.results[c]["out"].astype(np.float32)
    return out, res


def kernel(**inputs) -> np.ndarray:
    out, _ = _run(inputs, trace=False)
    return out
